# revision 18
# baseline (speedup 1.0000x reference)
"""nn_LphaLoss: full pipeline on 8 TRN2 NeuronCores via Bass/Tile.

Per core (SPMD, data-parallel over 128 of the 1024 32x32 blocks):
  VGG conv1_1..conv3_1 (bf16 matmuls, shifted-window conv), fft2 phase via
  DFT-matrix matmuls (fp32r), cosine sim of phases -> block mask,
  masked L1 partial sums. Host combines 8 (l1, count) pairs.

kernel(**inputs) takes FULL inputs, returns FULL (scalar) output.
"""
import numpy as np

BS = 32
THRESH = 0.2
MEAN = np.array([0.485, 0.456, 0.406], dtype=np.float32)
STD = np.array([0.229, 0.224, 0.225], dtype=np.float32)
N_CORES = 8
NP_FULL = 128          # block-pairs per core (1024 blocks / 8 cores)
G_FULL = 8             # pairs per group
PI = float(np.pi)

_COMPILED = {}
_WARMED = set()
LAST_EXEC_NS = None


# ---------------------------------------------------------------- device ----
def _build_device_kernel(NP, G):
    import concourse.bass as bass  # noqa: F401
    import concourse.mybir as mybir
    from concourse import bacc, bass_isa
    from concourse.tile import TileContext

    F32 = mybir.dt.float32
    F32R = mybir.dt.float32r
    BF16 = mybir.dt.bfloat16
    ALU = mybir.AluOpType
    ACT = mybir.ActivationFunctionType

    NG = NP // G           # number of groups
    NI = 2 * G             # VGG items per group (G pred1 + G target)

    nc = bacc.Bacc("TRN2", target_bir_lowering=False)
    # inputs (per core)
    x1_d = nc.declare_dram_parameter("x1", [3, NP, 32, 32], BF16, isOutput=False)
    xt_d = nc.declare_dram_parameter("xt", [3, NP, 32, 32], BF16, isOutput=False)
    x2_d = nc.declare_dram_parameter("x2", [NP, 3072], BF16, isOutput=False)
    w1s_d = nc.declare_dram_parameter("w1s", [3, 9, 64], BF16, isOutput=False)
    w2s_d = nc.declare_dram_parameter("w2s", [64, 9, 64], BF16, isOutput=False)
    w3s_d = nc.declare_dram_parameter("w3s", [64, 9, 128], BF16, isOutput=False)
    w4s_d = nc.declare_dram_parameter("w4s", [128, 9, 128], BF16, isOutput=False)
    w5s_d = nc.declare_dram_parameter("w5s", [128, 9, 256], BF16, isOutput=False)
    b1_d = nc.declare_dram_parameter("b1", [64, 1], F32, isOutput=False)
    b2_d = nc.declare_dram_parameter("b2", [64, 1], F32, isOutput=False)
    b3_d = nc.declare_dram_parameter("b3", [128, 1], F32, isOutput=False)
    b4_d = nc.declare_dram_parameter("b4", [128, 1], F32, isOutput=False)
    b5_d = nc.declare_dram_parameter("b5", [1, 256], F32, isOutput=False)
    mrt_d = nc.declare_dram_parameter("mrt", [64, 64], F32, isOutput=False)
    mit_d = nc.declare_dram_parameter("mit", [64, 64], F32, isOutput=False)
    ns_d = nc.declare_dram_parameter("ns", [3, 1], F32, isOutput=False)
    nb_d = nc.declare_dram_parameter("nb", [3, 1], F32, isOutput=False)
    o_d = nc.declare_dram_parameter("o", [1, 2], F32, isOutput=True)

    SH = [(dy, dx) for dy in range(3) for dx in range(3)]

    with TileContext(nc) as tc:
        with (
            tc.tile_pool(name="const", bufs=1) as cp,
            tc.tile_pool(name="feat", bufs=1) as fp,
            tc.tile_pool(name="work", bufs=2) as wp,
            tc.tile_pool(name="work1", bufs=1) as wp1,
            tc.tile_pool(name="mm", bufs=3, space="PSUM") as psA,
            tc.tile_pool(name="fftp", bufs=4, space="PSUM") as psF,
        ):
            # ---- constants ----
            wt1 = cp.tile_from(w1s_d[:, :, :])
            nst = cp.tile_from(ns_d[:, :])
            nbt = cp.tile_from(nb_d[:, :])
            wt2 = cp.tile_from(w2s_d[:, :, :])
            wt3 = cp.tile_from(w3s_d[:, :, :])
            wt4 = cp.tile_from(w4s_d[:, :, :])
            wt5 = cp.tile_from(w5s_d[:, :, :])
            b1t = cp.tile_from(b1_d[:, :])
            b2t = cp.tile_from(b2_d[:, :])
            b3t = cp.tile_from(b3_d[:, :])
            b4t = cp.tile_from(b4_d[:, :])
            b5r = cp.tile_from(b5_d[:, :])
            mrt = cp.tile_from(mrt_d[:, :])
            mit = cp.tile_from(mit_d[:, :])
            b5bc = cp.tile([64, 256], F32, tag="b5bc")
            nc.gpsimd.partition_broadcast(b5bc[:, :], b5r[:, :], channels=64)

            # ---- persistent feature buffers (zero borders once) ----
            pad0 = fp.tile([3, NI, 34, 34], BF16, tag="pad0")
            pad1 = fp.tile([64, NI, 34, 34], BF16, tag="pad1")
            pad2 = fp.tile([64, NI, 18, 18], BF16, tag="pad2")
            pad22 = fp.tile([128, NI, 18, 18], BF16, tag="pad22")
            pad3 = fp.tile([128, NI, 10, 10], BF16, tag="pad3")
            c3w = fp.tile([128, NI, 9, 64], BF16, tag="c3w")
            fftin1 = fp.tile([64, G * 256], F32, tag="fftin1")
            fftin2 = fp.tile([64, G * 256], F32, tag="fftin2")
            nc.vector.memset(pad0[:, :, :, :], 0.0)
            nc.vector.memset(pad1[:, :, :, :], 0.0)
            nc.vector.memset(pad2[:, :, :, :], 0.0)
            nc.vector.memset(pad22[:, :, :, :], 0.0)
            nc.vector.memset(pad3[:, :, :, :], 0.0)

            # ---- per-block results ----
            sums3 = cp.tile([64, 3, NP], F32, tag="sums3")
            l1blk = cp.tile([NP, 1], F32, tag="l1blk")
            maskT = cp.tile([NP, 1], F32, tag="maskT")
            stack2 = cp.tile([NP, 2], F32, tag="stack2")
            red2 = cp.tile([NP, 2], F32, tag="red2")

            # ---- L1 path (independent of mask until the end) ----
            p2t = cp.tile([NP, 3072], BF16, tag="p2t")
            tgt = cp.tile([NP, 3072], BF16, tag="tgt")
            nc.sync.dma_start(p2t[:, :], x2_d[:, :])
            tgv = tgt.rearrange("n (c h w) -> n c h w", c=3, h=32, w=32)
            for c in range(3):
                nc.sync.dma_start(tgv[:, c, :, :], xt_d[c, :, :, :])
            nc.vector.tensor_tensor(out=p2t[:, :], in0=p2t[:, :], in1=tgt[:, :],
                                    op=ALU.subtract)
            nc.vector.tensor_reduce(l1blk[:, :], p2t[:, :],
                                    axis=mybir.AxisListType.X, op=ALU.add,
                                    apply_absolute_value=True)

            # ---- main loop over groups ----
            for g in range(NG):
                g0 = g * G
                # conv1_1: stage raw blocks into pad0, normalize in place
                for j in range(NI):
                    srcd = x1_d if j < G else xt_d
                    i = g0 + (j if j < G else j - G)
                    nc.sync.dma_start(pad0[:, j, 1:33, 1:33], srcd[:, i, :, :])
                nc.vector.tensor_scalar(
                    out=pad0[:, :, 1:33, 1:33], in0=pad0[:, :, 1:33, 1:33],
                    scalar1=nst[:, :], scalar2=nbt[:, :],
                    op0=ALU.mult, op1=ALU.add)
                for j in range(NI):
                    for h in range(2):
                        ps = psA.tile([128, 512], F32, tag="mm")
                        for k, (dy, dx) in enumerate(SH):
                            nc.tensor.matmul(
                                ps[0:64, :], wt1[:, k, :],
                                pad0[:, j, 16 * h + dy:16 * h + dy + 16,
                                     dx:dx + 32],
                                start=(k == 0), stop=(k == 8))
                        nc.scalar.activation(
                            pad1[0:64, j, 1 + 16 * h:17 + 16 * h, 1:33],
                            ps[0:64, :].rearrange("p (h w) -> p h w", h=16, w=32),
                            ACT.Relu, bias=b1t[:, :], scale=1.0)

                # conv1_2 + pool1
                for j in range(NI):
                    for h in range(2):
                        ps = psA.tile([128, 512], F32, tag="mm")
                        for k, (dy, dx) in enumerate(SH):
                            nc.tensor.matmul(
                                ps[0:64, :],
                                wt2[:, k, :],
                                pad1[0:64, j, 16 * h + dy:16 * h + dy + 16,
                                     dx:dx + 32],
                                start=(k == 0), stop=(k == 8))
                        pv = ps.rearrange("p (a h b w) -> p a h b w",
                                          a=8, h=2, b=16, w=2)
                        t1 = wp.tile([64, 8, 16], F32, tag="pool1a")
                        nc.vector.tensor_copy(t1[:, :, :], pv[0:64, :, 0, :, 0])
                        for (hh, ww) in ((0, 1), (1, 0), (1, 1)):
                            nc.vector.tensor_tensor(
                                out=t1[:, :, :], in0=t1[:, :, :],
                                in1=pv[0:64, :, hh, :, ww], op=ALU.max)
                        nc.scalar.activation(
                            pad2[0:64, j, 1 + 8 * h:9 + 8 * h, 1:17],
                            t1[:, :, :], ACT.Relu, bias=b2t[:, :], scale=1.0)

                # conv2_1
                for j in range(NI):
                    ps = psA.tile([128, 512], F32, tag="mm")
                    for k, (dy, dx) in enumerate(SH):
                        nc.tensor.matmul(
                            ps[:, 0:256], wt3[:, k, :],
                            pad2[0:64, j, dy:dy + 16, dx:dx + 16],
                            start=(k == 0), stop=(k == 8))
                    nc.scalar.activation(
                        pad22[:, j, 1:17, 1:17],
                        ps[:, 0:256].rearrange("p (h w) -> p h w", h=16, w=16),
                        ACT.Relu, bias=b3t[:, :], scale=1.0)

                # conv2_2 + pool2
                for j in range(NI):
                    ps = psA.tile([128, 512], F32, tag="mm")
                    for k, (dy, dx) in enumerate(SH):
                        nc.tensor.matmul(
                            ps[:, 0:256], wt4[:, k, :],
                            pad22[:, j, dy:dy + 16, dx:dx + 16],
                            start=(k == 0), stop=(k == 8))
                    pv = ps[:, 0:256].rearrange("p (a h b w) -> p a h b w",
                                                a=8, h=2, b=8, w=2)
                    t1 = wp.tile([128, 8, 8], F32, tag="pool2a")
                    nc.vector.tensor_copy(t1[:, :, :], pv[:, :, 0, :, 0])
                    for (hh, ww) in ((0, 1), (1, 0), (1, 1)):
                        nc.vector.tensor_tensor(
                            out=t1[:, :, :], in0=t1[:, :, :],
                            in1=pv[:, :, hh, :, ww], op=ALU.max)
                    nc.scalar.activation(
                        pad3[:, j, 1:9, 1:9], t1[:, :, :],
                        ACT.Relu, bias=b4t[:, :], scale=1.0)

                # conv3_1 (stationary = shifted feature window, moving = W5)
                # stationary APs need one free dim -> stage contiguous windows
                for k, (dy, dx) in enumerate(SH):
                    nc.vector.tensor_copy(
                        c3w[:, :, k, :].rearrange("p n (h w) -> p n h w",
                                                  h=8, w=8),
                        pad3[:, :, dy:dy + 8, dx:dx + 8])
                for j in range(NI):
                    ps3 = psA.tile([128, 256], F32, tag="mm")
                    for k in range(9):
                        nc.tensor.matmul(
                            ps3[0:64, :], c3w[:, j, k, :],
                            wt5[:, k, :], start=(k == 0), stop=(k == 8))
                    dst = fftin1 if j < G else fftin2
                    i = j if j < G else j - G
                    nc.vector.tensor_tensor(
                        out=dst[:, i * 256:(i + 1) * 256], in0=ps3[0:64, :],
                        in1=b5bc[:, :], op=ALU.add)

                # fft2 phase + cosine-sim sums
                ph1 = wp1.tile([64, G * 256], F32, tag="ph1")
                ph2 = wp1.tile([64, G * 256], F32, tag="ph2")
                for src, ph in ((fftin1, ph1), (fftin2, ph2)):
                    for c0 in range(0, G * 256, 512):
                        sl = slice(c0, c0 + 512)
                        xr = psF.tile([64, 512], F32, tag="fft")
                        xi = psF.tile([64, 512], F32, tag="fft")
                        nc.tensor.matmul(xr[:, :], mrt[:, :], src[:, sl],
                                         start=True, stop=True)
                        nc.tensor.matmul(xi[:, :], mit[:, :], src[:, sl],
                                         start=True, stop=True)
                        # atan2 via octant reduction (Arctan LUT needs |in|<=pi/2):
                        # t = min(|xi|,|xr|)/max(...) in [0,1]; b = atan(t);
                        # if |xi|>|xr|: b = pi/2 - b;
                        # phi = sign(xi) * (pi*(xr<0) + (1-2*(xr<0))*b)
                        ax = wp1.tile([64, 512], F32, tag="ax")
                        ay = wp1.tile([64, 512], F32, tag="ay")
                        mx = wp1.tile([64, 512], F32, tag="mx")
                        mn = wp1.tile([64, 512], F32, tag="mn")
                        tt = wp1.tile([64, 512], F32, tag="tt")
                        bb = wp1.tile([64, 512], F32, tag="bb")
                        s1 = wp1.tile([64, 512], F32, tag="s1")
                        nn = wp1.tile([64, 512], F32, tag="nn")
                        nc.scalar.activation(ax[:, :], xr[:, :], ACT.Abs)
                        nc.scalar.activation(ay[:, :], xi[:, :], ACT.Abs)
                        nc.vector.tensor_tensor(out=mx[:, :], in0=ax[:, :],
                                                in1=ay[:, :], op=ALU.max)
                        nc.vector.tensor_tensor(out=mn[:, :], in0=ax[:, :],
                                                in1=ay[:, :], op=ALU.min)
                        nc.vector.reciprocal(tt[:, :], mx[:, :])
                        nc.vector.tensor_tensor(out=tt[:, :], in0=mn[:, :],
                                                in1=tt[:, :], op=ALU.mult)
                        nc.scalar.activation(bb[:, :], tt[:, :], ACT.Arctan)
                        # fold: bb += (ay>ax) * (pi/2 - 2*bb)
                        nc.vector.tensor_tensor(out=s1[:, :], in0=ay[:, :],
                                                in1=ax[:, :], op=ALU.is_gt)
                        nc.vector.tensor_scalar(
                            out=tt[:, :], in0=bb[:, :], scalar1=-2.0,
                            scalar2=PI / 2, op0=ALU.mult, op1=ALU.add)
                        nc.vector.tensor_tensor(out=s1[:, :], in0=s1[:, :],
                                                in1=tt[:, :], op=ALU.mult)
                        nc.vector.tensor_tensor(out=bb[:, :], in0=bb[:, :],
                                                in1=s1[:, :], op=ALU.add)
                        # quadrant: a4 = pi*n + (1-2n)*bb ; phi = sgn(xi)*a4
                        nc.vector.tensor_scalar(
                            out=nn[:, :], in0=xr[:, :], scalar1=0.0,
                            scalar2=None, op0=ALU.is_lt)
                        nc.vector.tensor_scalar(
                            out=tt[:, :], in0=nn[:, :], scalar1=-2.0,
                            scalar2=1.0, op0=ALU.mult, op1=ALU.add)
                        nc.vector.tensor_tensor(out=tt[:, :], in0=tt[:, :],
                                                in1=bb[:, :], op=ALU.mult)
                        nc.vector.tensor_scalar(
                            out=nn[:, :], in0=nn[:, :], scalar1=PI,
                            scalar2=None, op0=ALU.mult)
                        nc.vector.tensor_tensor(out=tt[:, :], in0=tt[:, :],
                                                in1=nn[:, :], op=ALU.add)
                        nc.vector.tensor_scalar(
                            out=nn[:, :], in0=xi[:, :], scalar1=0.0,
                            scalar2=-2.0, op0=ALU.is_lt, op1=ALU.mult)
                        nc.vector.tensor_scalar(
                            out=nn[:, :], in0=nn[:, :], scalar1=1.0,
                            scalar2=None, op0=ALU.add)
                        nc.vector.tensor_tensor(out=ph[:, sl], in0=tt[:, :],
                                                in1=nn[:, :], op=ALU.mult)
                prod = wp1.tile([64, G * 256], F32, tag="prod")
                pv1 = ph1.rearrange("p (i c) -> p i c", i=G, c=256)
                pv2 = ph2.rearrange("p (i c) -> p i c", i=G, c=256)
                pvp = prod.rearrange("p (i c) -> p i c", i=G, c=256)
                nc.vector.tensor_tensor(out=prod[:, :], in0=ph1[:, :],
                                        in1=ph2[:, :], op=ALU.mult)
                nc.vector.tensor_reduce(sums3[:, 0, g0:g0 + G], pvp[:, :, :],
                                        axis=mybir.AxisListType.X, op=ALU.add)
                nc.vector.tensor_tensor(out=prod[:, :], in0=ph1[:, :],
                                        in1=ph1[:, :], op=ALU.mult)
                nc.vector.tensor_reduce(sums3[:, 1, g0:g0 + G], pvp[:, :, :],
                                        axis=mybir.AxisListType.X, op=ALU.add)
                nc.vector.tensor_tensor(out=prod[:, :], in0=ph2[:, :],
                                        in1=ph2[:, :], op=ALU.mult)
                nc.vector.tensor_reduce(sums3[:, 2, g0:g0 + G], pvp[:, :, :],
                                        axis=mybir.AxisListType.X, op=ALU.add)

            # ---- finale: mask + masked L1 ----
            sums3r = cp.tile([64, 3, NP], F32, tag="sums3r")
            nc.gpsimd.partition_all_reduce(
                sums3r[:, :, :], sums3[:, :, :], channels=64,
                reduce_op=bass_isa.ReduceOp.add)
            num = sums3r[0:1, 0, :]
            nn1 = sums3r[0:1, 1, :]
            nn2 = sums3r[0:1, 2, :]
            ta = cp.tile([1, NP], F32, tag="ta")
            tb = cp.tile([1, NP], F32, tag="tb")
            mask = cp.tile([1, NP], F32, tag="mask")
            # mask = (num > 0) * (num^2 >= THRESH^2 * n1 * n2)
            nc.vector.tensor_tensor(out=ta[:, :], in0=nn1, in1=nn2,
                                    op=ALU.mult)
            nc.vector.tensor_scalar(out=ta[:, :], in0=ta[:, :],
                                    scalar1=float(THRESH * THRESH),
                                    scalar2=None, op0=ALU.mult)
            nc.vector.tensor_tensor(out=tb[:, :], in0=num, in1=num,
                                    op=ALU.mult)
            nc.vector.tensor_tensor(out=tb[:, :], in0=tb[:, :], in1=ta[:, :],
                                    op=ALU.is_ge)
            nc.vector.tensor_scalar(out=ta[:, :], in0=num, scalar1=0.0,
                                    scalar2=None, op0=ALU.is_gt)
            nc.vector.tensor_tensor(out=mask[:, :], in0=ta[:, :],
                                    in1=tb[:, :], op=ALU.mult)
            nc.sync.dma_start(maskT[:, :], mask[:, :])
            nc.vector.tensor_tensor(out=stack2[:, 0:1], in0=l1blk[:, :],
                                    in1=maskT[:, :], op=ALU.mult)
            nc.vector.tensor_copy(stack2[:, 1:2], maskT[:, :])
            nc.gpsimd.partition_all_reduce(
                red2[:, :], stack2[:, :], channels=NP,
                reduce_op=bass_isa.ReduceOp.add)
            nc.sync.dma_start(o_d[:, :], red2[0:1, :])
    nc.compile()
    return nc


# ------------------------------------------------------------------ host ----
def _blocks(x, B, C, nby, nbx):
    return (x.reshape(B, C, nby, BS, nbx, BS)
             .transpose(0, 2, 4, 1, 3, 5)
             .reshape(B * nby * nbx, C, BS, BS))


def _dft_mats():
    k = np.arange(8)
    ang = 2 * np.pi * np.outer(k, k) / 8.0
    C = np.cos(ang)
    S = np.sin(ang)
    S[0, :] = 0; S[4, :] = 0; S[:, 0] = 0; S[:, 4] = 0
    MR = (np.kron(C, C) - np.kron(S, S)).astype(np.float32)
    MI = (-(np.kron(C, S) + np.kron(S, C))).astype(np.float32)
    return MR.T.copy(), MI.T.copy()          # [pix, freq] = lhsT


def _pack_weights(w1, b1, w2, b2, w3, b3, w4, b4, w5, b5):
    import ml_dtypes
    bf16 = ml_dtypes.bfloat16
    def shifts(w):   # [O, C, 3, 3] -> [C, 9, O]
        return np.ascontiguousarray(w.transpose(1, 2, 3, 0).reshape(
            w.shape[1], 9, w.shape[0]))
    mrt, mit = _dft_mats()
    return {
        "w1s": shifts(w1).astype(bf16),
        "ns": (1.0 / STD).reshape(3, 1).astype(np.float32),
        "nb": (-MEAN / STD).reshape(3, 1).astype(np.float32),
        "w2s": shifts(w2).astype(bf16), "w3s": shifts(w3).astype(bf16),
        "w4s": shifts(w4).astype(bf16), "w5s": shifts(w5).astype(bf16),
        "b1": b1.reshape(64, 1).astype(np.float32),
        "b2": b2.reshape(64, 1).astype(np.float32),
        "b3": b3.reshape(128, 1).astype(np.float32),
        "b4": b4.reshape(128, 1).astype(np.float32),
        "b5": b5.reshape(1, 256).astype(np.float32),
        "mrt": mrt, "mit": mit,
    }


def _make_in_maps(pred1, pred2, target, params, NP):
    import ml_dtypes
    bf16 = ml_dtypes.bfloat16
    B, C, H, W = pred1.shape
    nby, nbx = H // BS, W // BS
    N = B * nby * nbx
    wts = _pack_weights(*params)
    x1b = _blocks(pred1, B, C, nby, nbx).transpose(1, 0, 2, 3)  # [3,N,32,32]
    xtb = _blocks(target, B, C, nby, nbx).transpose(1, 0, 2, 3)
    x2b = _blocks(pred2, B, C, nby, nbx).reshape(N, 3072)
    x1b = np.ascontiguousarray(x1b).astype(bf16)
    xtb = np.ascontiguousarray(xtb).astype(bf16)
    x2b = np.ascontiguousarray(x2b).astype(bf16)
    in_maps = []
    for c in range(N // NP):
        s = slice(c * NP, (c + 1) * NP)
        m = {"x1": np.ascontiguousarray(x1b[:, s]),
             "xt": np.ascontiguousarray(xtb[:, s]),
             "x2": np.ascontiguousarray(x2b[s])}
        m.update(wts)
        in_maps.append(m)
    return in_maps


def kernel(pred1, pred2, target, w1, b1, w2, b2, w3, b3, w4, b4, w5, b5):
    import time as _time
    from concourse.bass_utils import run_bass_kernel_spmd
    from concourse import bass2jax

    pred1 = np.asarray(pred1, dtype=np.float32)
    pred2 = np.asarray(pred2, dtype=np.float32)
    target = np.asarray(target, dtype=np.float32)
    params = tuple(np.asarray(a, dtype=np.float32)
                   for a in (w1, b1, w2, b2, w3, b3, w4, b4, w5, b5))

    NP = NP_FULL
    key = (NP, G_FULL)
    if key not in _COMPILED:
        _COMPILED[key] = _build_device_kernel(NP, G_FULL)
    nc = _COMPILED[key]

    in_maps = _make_in_maps(pred1, pred2, target, params, NP)

    # warm the compile/jit caches outside the timed call (same HLO; results
    # discarded) so the timed run measures transfer + execution only.
    if key not in _WARMED:
        zero_maps = [{k: np.zeros_like(v) for k, v in in_maps[0].items()}
                     for _ in range(N_CORES)]
        bass2jax.run_bass_via_pjrt(nc, zero_maps, n_cores=N_CORES)
        _WARMED.add(key)

    t0 = _time.perf_counter()
    res = run_bass_kernel_spmd(nc, in_maps, list(range(N_CORES)))
    global LAST_EXEC_NS
    LAST_EXEC_NS = int((_time.perf_counter() - t0) * 1e9)
    if res.exec_time_ns:
        LAST_EXEC_NS = int(res.exec_time_ns)

    l1_total = np.float64(0.0)
    cnt_total = np.float64(0.0)
    for c in range(N_CORES):
        o = res.results[c]["o"]
        l1_total += np.float64(o[0, 0])
        cnt_total += np.float64(o[0, 1])
    mask_sum = cnt_total * (BS * BS)
    out = l1_total / (mask_sum + 1e-6)
    return np.array(out, dtype=np.float32)


# revision 19
# speedup vs baseline: 4.9848x; 4.9848x over previous
"""nn_LphaLoss: full pipeline on 8 TRN2 NeuronCores via Bass/Tile.

Per core (SPMD, data-parallel over 128 of the 1024 32x32 blocks):
  VGG conv1_1..conv3_1 (bf16 matmuls, shifted-window conv), fft2 phase via
  DFT-matrix matmuls (fp32r), cosine sim of phases -> block mask,
  masked L1 partial sums. Host combines 8 (l1, count) pairs.

kernel(**inputs) takes FULL inputs, returns FULL (scalar) output.
"""
import numpy as np

BS = 32
THRESH = 0.2
MEAN = np.array([0.485, 0.456, 0.406], dtype=np.float32)
STD = np.array([0.229, 0.224, 0.225], dtype=np.float32)
N_CORES = 8
NP_FULL = 128          # block-pairs per core (1024 blocks / 8 cores)
G_FULL = 8             # pairs per group
PI = float(np.pi)

_COMPILED = {}
_WARMED = set()
LAST_EXEC_NS = None


# ---------------------------------------------------------------- device ----
def _build_device_kernel(NP, G):
    import concourse.bass as bass
    import concourse.mybir as mybir
    from concourse import bacc, bass_isa
    from concourse.tile import TileContext

    F32 = mybir.dt.float32
    F32R = mybir.dt.float32r
    BF16 = mybir.dt.bfloat16
    ALU = mybir.AluOpType
    ACT = mybir.ActivationFunctionType

    NG = NP // G           # number of groups
    NI = 2 * G             # VGG items per group (G pred1 + G target)

    nc = bacc.Bacc("TRN2", target_bir_lowering=False)
    # inputs (per core)
    x1_d = nc.declare_dram_parameter("x1", [3, NP, 32, 32], BF16, isOutput=False)
    xt_d = nc.declare_dram_parameter("xt", [3, NP, 32, 32], BF16, isOutput=False)
    x2_d = nc.declare_dram_parameter("x2", [NP, 3072], BF16, isOutput=False)
    w1s_d = nc.declare_dram_parameter("w1s", [3, 9, 64], BF16, isOutput=False)
    w2s_d = nc.declare_dram_parameter("w2s", [64, 9, 64], BF16, isOutput=False)
    w3s_d = nc.declare_dram_parameter("w3s", [64, 9, 128], BF16, isOutput=False)
    w4s_d = nc.declare_dram_parameter("w4s", [128, 9, 128], BF16, isOutput=False)
    w5s_d = nc.declare_dram_parameter("w5s", [128, 9, 256], BF16, isOutput=False)
    b1_d = nc.declare_dram_parameter("b1", [64, 1], F32, isOutput=False)
    b2_d = nc.declare_dram_parameter("b2", [64, 1], F32, isOutput=False)
    b3_d = nc.declare_dram_parameter("b3", [128, 1], F32, isOutput=False)
    b4_d = nc.declare_dram_parameter("b4", [128, 1], F32, isOutput=False)
    b5_d = nc.declare_dram_parameter("b5", [1, 256], F32, isOutput=False)
    mrt_d = nc.declare_dram_parameter("mrt", [64, 64], F32, isOutput=False)
    mit_d = nc.declare_dram_parameter("mit", [64, 64], F32, isOutput=False)
    ns_d = nc.declare_dram_parameter("ns", [3, 1], F32, isOutput=False)
    nb_d = nc.declare_dram_parameter("nb", [3, 1], F32, isOutput=False)
    o_d = nc.declare_dram_parameter("o", [1, 2], F32, isOutput=True)

    SH = [(dy, dx) for dy in range(3) for dx in range(3)]
    sums_d = nc.dram_tensor("sums_i", [64, 3, NG, G], F32, kind="Internal")

    with TileContext(nc) as tc:
        with (
            tc.tile_pool(name="const", bufs=1) as cp,
            tc.tile_pool(name="feat", bufs=1) as fp,
            tc.tile_pool(name="work", bufs=2) as wp,
            tc.tile_pool(name="work1", bufs=1) as wp1,
            tc.tile_pool(name="mm", bufs=3, space="PSUM") as psA,
            tc.tile_pool(name="fftp", bufs=4, space="PSUM") as psF,
        ):
            # ---- constants ----
            wt1 = cp.tile_from(w1s_d[:, :, :])
            nst = cp.tile_from(ns_d[:, :])
            nbt = cp.tile_from(nb_d[:, :])
            wt2 = cp.tile_from(w2s_d[:, :, :])
            wt3 = cp.tile_from(w3s_d[:, :, :])
            wt4 = cp.tile_from(w4s_d[:, :, :])
            wt5 = cp.tile_from(w5s_d[:, :, :])
            b1t = cp.tile_from(b1_d[:, :])
            b2t = cp.tile_from(b2_d[:, :])
            b3t = cp.tile_from(b3_d[:, :])
            b4t = cp.tile_from(b4_d[:, :])
            b5r = cp.tile_from(b5_d[:, :])
            mrt = cp.tile_from(mrt_d[:, :])
            mit = cp.tile_from(mit_d[:, :])
            b5bc = cp.tile([64, 256], F32, tag="b5bc")
            nc.gpsimd.partition_broadcast(b5bc[:, :], b5r[:, :], channels=64)

            # ---- persistent feature buffers (zero borders once) ----
            pad0 = fp.tile([3, NI, 34, 34], BF16, tag="pad0")
            pad1 = fp.tile([64, NI, 34, 34], BF16, tag="pad1")
            pad2 = fp.tile([64, NI, 18, 18], BF16, tag="pad2")
            pad22 = fp.tile([128, NI, 18, 18], BF16, tag="pad22")
            pad3 = fp.tile([128, NI, 10, 10], BF16, tag="pad3")
            c3w = fp.tile([128, NI, 9, 64], BF16, tag="c3w")
            fftin1 = fp.tile([64, G * 256], F32, tag="fftin1")
            fftin2 = fp.tile([64, G * 256], F32, tag="fftin2")
            nc.vector.memset(pad0[:, :, :, :], 0.0)
            nc.vector.memset(pad1[:, :, :, :], 0.0)
            nc.vector.memset(pad2[:, :, :, :], 0.0)
            nc.vector.memset(pad22[:, :, :, :], 0.0)
            nc.vector.memset(pad3[:, :, :, :], 0.0)

            # ---- per-block results ----
            sums3 = cp.tile([64, 3, NP], F32, tag="sums3")
            l1blk = cp.tile([NP, 1], F32, tag="l1blk")
            maskT = cp.tile([NP, 1], F32, tag="maskT")
            stack2 = cp.tile([NP, 2], F32, tag="stack2")
            red2 = cp.tile([NP, 2], F32, tag="red2")

            # ---- L1 path (independent of mask until the end) ----
            p2t = cp.tile([NP, 3072], BF16, tag="p2t")
            tgt = cp.tile([NP, 3072], BF16, tag="tgt")
            nc.sync.dma_start(p2t[:, :], x2_d[:, :])
            tgv = tgt.rearrange("n (c h w) -> n c h w", c=3, h=32, w=32)
            for c in range(3):
                nc.sync.dma_start(tgv[:, c, :, :], xt_d[c, :, :, :])
            nc.vector.tensor_tensor(out=p2t[:, :], in0=p2t[:, :], in1=tgt[:, :],
                                    op=ALU.subtract)
            nc.vector.tensor_reduce(l1blk[:, :], p2t[:, :],
                                    axis=mybir.AxisListType.X, op=ALU.add,
                                    apply_absolute_value=True)

            # ---- main loop over groups (hardware loop) ----
            x1v = x1_d.rearrange("c (ng g) h w -> c ng g h w", g=G)
            xtv = xt_d.rearrange("c (ng g) h w -> c ng g h w", g=G)
            with tc.For_i(0, NG, 1) as gi:
                # conv1_1: stage raw blocks into pad0, normalize in place
                for j in range(NI):
                    srcv = x1v if j < G else xtv
                    jj = j if j < G else j - G
                    nc.sync.dma_start(pad0[:, j, 1:33, 1:33],
                                      srcv[:, bass.ds(gi, 1), jj, :, :])
                nc.vector.tensor_scalar(
                    out=pad0[:, :, 1:33, 1:33], in0=pad0[:, :, 1:33, 1:33],
                    scalar1=nst[:, :], scalar2=nbt[:, :],
                    op0=ALU.mult, op1=ALU.add)
                for j in range(NI):
                    for h in range(2):
                        ps = psA.tile([128, 512], F32, tag="mm")
                        for k, (dy, dx) in enumerate(SH):
                            nc.tensor.matmul(
                                ps[0:64, :], wt1[:, k, :],
                                pad0[:, j, 16 * h + dy:16 * h + dy + 16,
                                     dx:dx + 32],
                                start=(k == 0), stop=(k == 8))
                        nc.scalar.activation(
                            pad1[0:64, j, 1 + 16 * h:17 + 16 * h, 1:33],
                            ps[0:64, :].rearrange("p (h w) -> p h w", h=16, w=32),
                            ACT.Relu, bias=b1t[:, :], scale=1.0)

                # conv1_2 + pool1
                for j in range(NI):
                    for h in range(2):
                        ps = psA.tile([128, 512], F32, tag="mm")
                        for k, (dy, dx) in enumerate(SH):
                            nc.tensor.matmul(
                                ps[0:64, :],
                                wt2[:, k, :],
                                pad1[0:64, j, 16 * h + dy:16 * h + dy + 16,
                                     dx:dx + 32],
                                start=(k == 0), stop=(k == 8))
                        pv = ps.rearrange("p (a h b w) -> p a h b w",
                                          a=8, h=2, b=16, w=2)
                        t1 = wp.tile([64, 8, 16], F32, tag="pool1a")
                        nc.vector.tensor_copy(t1[:, :, :], pv[0:64, :, 0, :, 0])
                        for (hh, ww) in ((0, 1), (1, 0), (1, 1)):
                            nc.vector.tensor_tensor(
                                out=t1[:, :, :], in0=t1[:, :, :],
                                in1=pv[0:64, :, hh, :, ww], op=ALU.max)
                        nc.scalar.activation(
                            pad2[0:64, j, 1 + 8 * h:9 + 8 * h, 1:17],
                            t1[:, :, :], ACT.Relu, bias=b2t[:, :], scale=1.0)

                # conv2_1
                for j in range(NI):
                    ps = psA.tile([128, 512], F32, tag="mm")
                    for k, (dy, dx) in enumerate(SH):
                        nc.tensor.matmul(
                            ps[:, 0:256], wt3[:, k, :],
                            pad2[0:64, j, dy:dy + 16, dx:dx + 16],
                            start=(k == 0), stop=(k == 8))
                    nc.scalar.activation(
                        pad22[:, j, 1:17, 1:17],
                        ps[:, 0:256].rearrange("p (h w) -> p h w", h=16, w=16),
                        ACT.Relu, bias=b3t[:, :], scale=1.0)

                # conv2_2 + pool2
                for j in range(NI):
                    ps = psA.tile([128, 512], F32, tag="mm")
                    for k, (dy, dx) in enumerate(SH):
                        nc.tensor.matmul(
                            ps[:, 0:256], wt4[:, k, :],
                            pad22[:, j, dy:dy + 16, dx:dx + 16],
                            start=(k == 0), stop=(k == 8))
                    pv = ps[:, 0:256].rearrange("p (a h b w) -> p a h b w",
                                                a=8, h=2, b=8, w=2)
                    t1 = wp.tile([128, 8, 8], F32, tag="pool2a")
                    nc.vector.tensor_copy(t1[:, :, :], pv[:, :, 0, :, 0])
                    for (hh, ww) in ((0, 1), (1, 0), (1, 1)):
                        nc.vector.tensor_tensor(
                            out=t1[:, :, :], in0=t1[:, :, :],
                            in1=pv[:, :, hh, :, ww], op=ALU.max)
                    nc.scalar.activation(
                        pad3[:, j, 1:9, 1:9], t1[:, :, :],
                        ACT.Relu, bias=b4t[:, :], scale=1.0)

                # conv3_1 (stationary = shifted feature window, moving = W5)
                # stationary APs need one free dim -> stage contiguous windows
                for k, (dy, dx) in enumerate(SH):
                    nc.vector.tensor_copy(
                        c3w[:, :, k, :].rearrange("p n (h w) -> p n h w",
                                                  h=8, w=8),
                        pad3[:, :, dy:dy + 8, dx:dx + 8])
                for j in range(NI):
                    ps3 = psA.tile([128, 256], F32, tag="mm")
                    for k in range(9):
                        nc.tensor.matmul(
                            ps3[0:64, :], c3w[:, j, k, :],
                            wt5[:, k, :], start=(k == 0), stop=(k == 8))
                    dst = fftin1 if j < G else fftin2
                    i = j if j < G else j - G
                    nc.vector.tensor_tensor(
                        out=dst[:, i * 256:(i + 1) * 256], in0=ps3[0:64, :],
                        in1=b5bc[:, :], op=ALU.add)

                # fft2 phase + cosine-sim sums
                ph1 = wp1.tile([64, G * 256], F32, tag="ph1")
                ph2 = wp1.tile([64, G * 256], F32, tag="ph2")
                for src, ph in ((fftin1, ph1), (fftin2, ph2)):
                    for c0 in range(0, G * 256, 512):
                        sl = slice(c0, c0 + 512)
                        xr = psF.tile([64, 512], F32, tag="fft")
                        xi = psF.tile([64, 512], F32, tag="fft")
                        nc.tensor.matmul(xr[:, :], mrt[:, :], src[:, sl],
                                         start=True, stop=True)
                        nc.tensor.matmul(xi[:, :], mit[:, :], src[:, sl],
                                         start=True, stop=True)
                        # atan2 via octant reduction (Arctan LUT needs |in|<=pi/2):
                        # t = min(|xi|,|xr|)/max(...) in [0,1]; b = atan(t);
                        # if |xi|>|xr|: b = pi/2 - b;
                        # phi = sign(xi) * (pi*(xr<0) + (1-2*(xr<0))*b)
                        ax = wp1.tile([64, 512], F32, tag="ax")
                        ay = wp1.tile([64, 512], F32, tag="ay")
                        mx = wp1.tile([64, 512], F32, tag="mx")
                        mn = wp1.tile([64, 512], F32, tag="mn")
                        tt = wp1.tile([64, 512], F32, tag="tt")
                        bb = wp1.tile([64, 512], F32, tag="bb")
                        s1 = wp1.tile([64, 512], F32, tag="s1")
                        nn = wp1.tile([64, 512], F32, tag="nn")
                        nc.scalar.activation(ax[:, :], xr[:, :], ACT.Abs)
                        nc.scalar.activation(ay[:, :], xi[:, :], ACT.Abs)
                        nc.vector.tensor_tensor(out=mx[:, :], in0=ax[:, :],
                                                in1=ay[:, :], op=ALU.max)
                        nc.vector.tensor_tensor(out=mn[:, :], in0=ax[:, :],
                                                in1=ay[:, :], op=ALU.min)
                        nc.vector.reciprocal(tt[:, :], mx[:, :])
                        nc.vector.tensor_tensor(out=tt[:, :], in0=mn[:, :],
                                                in1=tt[:, :], op=ALU.mult)
                        nc.scalar.activation(bb[:, :], tt[:, :], ACT.Arctan)
                        # fold: bb += (ay>ax) * (pi/2 - 2*bb)
                        nc.vector.tensor_tensor(out=s1[:, :], in0=ay[:, :],
                                                in1=ax[:, :], op=ALU.is_gt)
                        nc.vector.tensor_scalar(
                            out=tt[:, :], in0=bb[:, :], scalar1=-2.0,
                            scalar2=PI / 2, op0=ALU.mult, op1=ALU.add)
                        nc.vector.tensor_tensor(out=s1[:, :], in0=s1[:, :],
                                                in1=tt[:, :], op=ALU.mult)
                        nc.vector.tensor_tensor(out=bb[:, :], in0=bb[:, :],
                                                in1=s1[:, :], op=ALU.add)
                        # quadrant: a4 = pi*n + (1-2n)*bb ; phi = sgn(xi)*a4
                        nc.vector.tensor_scalar(
                            out=nn[:, :], in0=xr[:, :], scalar1=0.0,
                            scalar2=None, op0=ALU.is_lt)
                        nc.vector.tensor_scalar(
                            out=tt[:, :], in0=nn[:, :], scalar1=-2.0,
                            scalar2=1.0, op0=ALU.mult, op1=ALU.add)
                        nc.vector.tensor_tensor(out=tt[:, :], in0=tt[:, :],
                                                in1=bb[:, :], op=ALU.mult)
                        nc.vector.tensor_scalar(
                            out=nn[:, :], in0=nn[:, :], scalar1=PI,
                            scalar2=None, op0=ALU.mult)
                        nc.vector.tensor_tensor(out=tt[:, :], in0=tt[:, :],
                                                in1=nn[:, :], op=ALU.add)
                        nc.vector.tensor_scalar(
                            out=nn[:, :], in0=xi[:, :], scalar1=0.0,
                            scalar2=-2.0, op0=ALU.is_lt, op1=ALU.mult)
                        nc.vector.tensor_scalar(
                            out=nn[:, :], in0=nn[:, :], scalar1=1.0,
                            scalar2=None, op0=ALU.add)
                        nc.vector.tensor_tensor(out=ph[:, sl], in0=tt[:, :],
                                                in1=nn[:, :], op=ALU.mult)
                prod = wp1.tile([64, G * 256], F32, tag="prod")
                pvp = prod.rearrange("p (i c) -> p i c", i=G, c=256)
                s3g = wp1.tile([64, 3, G], F32, tag="s3g")
                nc.vector.tensor_tensor(out=prod[:, :], in0=ph1[:, :],
                                        in1=ph2[:, :], op=ALU.mult)
                nc.vector.tensor_reduce(s3g[:, 0, :], pvp[:, :, :],
                                        axis=mybir.AxisListType.X, op=ALU.add)
                nc.vector.tensor_tensor(out=prod[:, :], in0=ph1[:, :],
                                        in1=ph1[:, :], op=ALU.mult)
                nc.vector.tensor_reduce(s3g[:, 1, :], pvp[:, :, :],
                                        axis=mybir.AxisListType.X, op=ALU.add)
                nc.vector.tensor_tensor(out=prod[:, :], in0=ph2[:, :],
                                        in1=ph2[:, :], op=ALU.mult)
                nc.vector.tensor_reduce(s3g[:, 2, :], pvp[:, :, :],
                                        axis=mybir.AxisListType.X, op=ALU.add)
                nc.sync.dma_start(sums_d[:, :, bass.ds(gi, 1), :],
                                  s3g[:, :, :])

            # ---- finale: mask + masked L1 ----
            nc.sync.dma_start(
                sums3[:, :, :],
                sums_d.rearrange("p s ng g -> p s (ng g)")[:, :, :])
            sums3r = cp.tile([64, 3, NP], F32, tag="sums3r")
            nc.gpsimd.partition_all_reduce(
                sums3r[:, :, :], sums3[:, :, :], channels=64,
                reduce_op=bass_isa.ReduceOp.add)
            num = sums3r[0:1, 0, :]
            nn1 = sums3r[0:1, 1, :]
            nn2 = sums3r[0:1, 2, :]
            ta = cp.tile([1, NP], F32, tag="ta")
            tb = cp.tile([1, NP], F32, tag="tb")
            mask = cp.tile([1, NP], F32, tag="mask")
            # mask = (num > 0) * (num^2 >= THRESH^2 * n1 * n2)
            nc.vector.tensor_tensor(out=ta[:, :], in0=nn1, in1=nn2,
                                    op=ALU.mult)
            nc.vector.tensor_scalar(out=ta[:, :], in0=ta[:, :],
                                    scalar1=float(THRESH * THRESH),
                                    scalar2=None, op0=ALU.mult)
            nc.vector.tensor_tensor(out=tb[:, :], in0=num, in1=num,
                                    op=ALU.mult)
            nc.vector.tensor_tensor(out=tb[:, :], in0=tb[:, :], in1=ta[:, :],
                                    op=ALU.is_ge)
            nc.vector.tensor_scalar(out=ta[:, :], in0=num, scalar1=0.0,
                                    scalar2=None, op0=ALU.is_gt)
            nc.vector.tensor_tensor(out=mask[:, :], in0=ta[:, :],
                                    in1=tb[:, :], op=ALU.mult)
            nc.sync.dma_start(maskT[:, :], mask[:, :])
            nc.vector.tensor_tensor(out=stack2[:, 0:1], in0=l1blk[:, :],
                                    in1=maskT[:, :], op=ALU.mult)
            nc.vector.tensor_copy(stack2[:, 1:2], maskT[:, :])
            nc.gpsimd.partition_all_reduce(
                red2[:, :], stack2[:, :], channels=NP,
                reduce_op=bass_isa.ReduceOp.add)
            nc.sync.dma_start(o_d[:, :], red2[0:1, :])
    nc.compile()
    return nc


# ------------------------------------------------------------------ host ----
def _blocks(x, B, C, nby, nbx):
    return (x.reshape(B, C, nby, BS, nbx, BS)
             .transpose(0, 2, 4, 1, 3, 5)
             .reshape(B * nby * nbx, C, BS, BS))


def _dft_mats():
    k = np.arange(8)
    ang = 2 * np.pi * np.outer(k, k) / 8.0
    C = np.cos(ang)
    S = np.sin(ang)
    S[0, :] = 0; S[4, :] = 0; S[:, 0] = 0; S[:, 4] = 0
    MR = (np.kron(C, C) - np.kron(S, S)).astype(np.float32)
    MI = (-(np.kron(C, S) + np.kron(S, C))).astype(np.float32)
    return MR.T.copy(), MI.T.copy()          # [pix, freq] = lhsT


def _pack_weights(w1, b1, w2, b2, w3, b3, w4, b4, w5, b5):
    import ml_dtypes
    bf16 = ml_dtypes.bfloat16
    def shifts(w):   # [O, C, 3, 3] -> [C, 9, O]
        return np.ascontiguousarray(w.transpose(1, 2, 3, 0).reshape(
            w.shape[1], 9, w.shape[0]))
    mrt, mit = _dft_mats()
    return {
        "w1s": shifts(w1).astype(bf16),
        "ns": (1.0 / STD).reshape(3, 1).astype(np.float32),
        "nb": (-MEAN / STD).reshape(3, 1).astype(np.float32),
        "w2s": shifts(w2).astype(bf16), "w3s": shifts(w3).astype(bf16),
        "w4s": shifts(w4).astype(bf16), "w5s": shifts(w5).astype(bf16),
        "b1": b1.reshape(64, 1).astype(np.float32),
        "b2": b2.reshape(64, 1).astype(np.float32),
        "b3": b3.reshape(128, 1).astype(np.float32),
        "b4": b4.reshape(128, 1).astype(np.float32),
        "b5": b5.reshape(1, 256).astype(np.float32),
        "mrt": mrt, "mit": mit,
    }


def _make_in_maps(pred1, pred2, target, params, NP):
    import ml_dtypes
    bf16 = ml_dtypes.bfloat16
    B, C, H, W = pred1.shape
    nby, nbx = H // BS, W // BS
    N = B * nby * nbx
    wts = _pack_weights(*params)
    x1b = _blocks(pred1, B, C, nby, nbx).transpose(1, 0, 2, 3)  # [3,N,32,32]
    xtb = _blocks(target, B, C, nby, nbx).transpose(1, 0, 2, 3)
    x2b = _blocks(pred2, B, C, nby, nbx).reshape(N, 3072)
    x1b = np.ascontiguousarray(x1b).astype(bf16)
    xtb = np.ascontiguousarray(xtb).astype(bf16)
    x2b = np.ascontiguousarray(x2b).astype(bf16)
    in_maps = []
    for c in range(N // NP):
        s = slice(c * NP, (c + 1) * NP)
        m = {"x1": np.ascontiguousarray(x1b[:, s]),
             "xt": np.ascontiguousarray(xtb[:, s]),
             "x2": np.ascontiguousarray(x2b[s])}
        m.update(wts)
        in_maps.append(m)
    return in_maps


def kernel(pred1, pred2, target, w1, b1, w2, b2, w3, b3, w4, b4, w5, b5):
    import time as _time
    from concourse.bass_utils import run_bass_kernel_spmd
    from concourse import bass2jax

    pred1 = np.asarray(pred1, dtype=np.float32)
    pred2 = np.asarray(pred2, dtype=np.float32)
    target = np.asarray(target, dtype=np.float32)
    params = tuple(np.asarray(a, dtype=np.float32)
                   for a in (w1, b1, w2, b2, w3, b3, w4, b4, w5, b5))

    NP = NP_FULL
    key = (NP, G_FULL)
    if key not in _COMPILED:
        _COMPILED[key] = _build_device_kernel(NP, G_FULL)
    nc = _COMPILED[key]

    in_maps = _make_in_maps(pred1, pred2, target, params, NP)

    # warm the compile/jit caches outside the timed call (same HLO; results
    # discarded) so the timed run measures transfer + execution only.
    if key not in _WARMED:
        try:
            import jax
            jax.config.update("jax_compilation_cache_dir",
                              "/root/.jax_bass_cache")
            jax.config.update("jax_persistent_cache_min_compile_time_secs", 0)
            jax.config.update("jax_persistent_cache_min_entry_size_bytes", 0)
        except Exception:
            pass
        zero_maps = [{k: np.zeros_like(v) for k, v in in_maps[0].items()}
                     for _ in range(N_CORES)]
        bass2jax.run_bass_via_pjrt(nc, zero_maps, n_cores=N_CORES)
        _WARMED.add(key)

    t0 = _time.perf_counter()
    res = run_bass_kernel_spmd(nc, in_maps, list(range(N_CORES)))
    global LAST_EXEC_NS
    LAST_EXEC_NS = int((_time.perf_counter() - t0) * 1e9)
    if res.exec_time_ns:
        LAST_EXEC_NS = int(res.exec_time_ns)

    l1_total = np.float64(0.0)
    cnt_total = np.float64(0.0)
    for c in range(N_CORES):
        o = res.results[c]["o"]
        l1_total += np.float64(o[0, 0])
        cnt_total += np.float64(o[0, 1])
    mask_sum = cnt_total * (BS * BS)
    out = l1_total / (mask_sum + 1e-6)
    return np.array(out, dtype=np.float32)
